# revision 4
# baseline (speedup 1.0000x reference)
"""Multi-head attention kernel for 8 TRN2 NeuronCores.

Problem: B=2, S=2048, D=1024, H=16 heads, head_dim=64, fp32 I/O.

Sharding (per the tensor-parallel hint): 8 cores = 2 batches x 4 head-groups.
Core c handles batch c//4 and heads [4*(c%4), 4*(c%4)+4). Each core:
  - projects its head-slice qT/kT (feature-on-partition layout, 2 heads per
    128-partition tile) and v (natural layout, with an appended ones column),
  - computes scoresT = k @ q.T per head with K=64 row-tiled matmuls (two heads
    run concurrently in the PE array),
  - exp on ScalarE with the 1/sqrt(64) scale and the -1e9 mask folded into the
    activation's scale/bias,
  - attn@v with the [v|1] trick: the ones column makes the softmax denominator
    fall out of the same matmul stream (PSUM row 64),
  - normalizes via reciprocal + a rank-1 PE broadcast matmul,
  - computes a partial output projection over its 256 features.
Host sums the 4 partials per batch and adds the output bias.
All matmul operands are bf16 (fp32 matmul is 4x slower on the PE array);
accumulation is fp32 in PSUM and the returned partials are fp32.
"""

import numpy as np
import ml_dtypes

import concourse.bass as bass
import concourse.mybir as mybir
import concourse.tile as tile
from concourse import bacc
from concourse.bass_utils import run_bass_kernel_spmd

BF16 = mybir.dt.bfloat16
FP32 = mybir.dt.float32

B, S, D = 2, 2048, 1024
NH, DH = 16, 64
NCORES = 8
GROUPS = 4                 # head-groups (cores per batch)
HL = NH // GROUPS          # heads per core = 4
FL = HL * DH               # features per core = 256
NPAIR = HL // 2            # head pairs per core = 2

SC = 512                   # i/s chunk (PSUM bank = 512 fp32)
JC = 128                   # j chunk (partition dim)
DCH = D // 128             # contraction chunks over embed dim = 8
N_SC = S // SC             # 4
N_JC = S // JC             # 16


def build_kernel():
    nc = bacc.Bacc("TRN2", target_bir_lowering=False, debug=False)

    xT = nc.dram_tensor("xT", [D, S], BF16, kind="ExternalInput")
    wq = nc.dram_tensor("wq", [D, FL], BF16, kind="ExternalInput")
    wk = nc.dram_tensor("wk", [D, FL], BF16, kind="ExternalInput")
    wv = nc.dram_tensor("wv", [D, FL], BF16, kind="ExternalInput")
    wo = nc.dram_tensor("wo", [FL, D], BF16, kind="ExternalInput")
    bqc = nc.dram_tensor("bqc", [128, 2], FP32, kind="ExternalInput")
    bkc = nc.dram_tensor("bkc", [128, 2], FP32, kind="ExternalInput")
    bvb = nc.dram_tensor("bvb", [128, FL], FP32, kind="ExternalInput")
    mb = nc.dram_tensor("mb", [128, N_JC], FP32, kind="ExternalInput")
    out = nc.dram_tensor("out", [S, D], FP32, kind="ExternalOutput")

    with tile.TileContext(nc) as tc:
        with (
            tc.tile_pool(name="weights", bufs=1) as wpool,
            tc.tile_pool(name="acts", bufs=1) as apool,
            tc.tile_pool(name="exps", bufs=4) as epool,
            tc.tile_pool(name="stages", bufs=4) as spool,
            tc.tile_pool(name="smalls", bufs=2) as smpool,
            tc.tile_pool(name="scores", bufs=2, space="PSUM") as scpool,
            tc.tile_pool(name="attnout", bufs=2, space="PSUM") as aopool,
            tc.tile_pool(name="projacc", bufs=2, space="PSUM") as prpool,
        ):
            # ---- resident inputs ----
            xt = []
            for dc in range(DCH):
                t = wpool.tile([128, S], BF16, name=f"xt{dc}")
                nc.sync.dma_start(out=t, in_=xT.ap()[dc * 128:(dc + 1) * 128, :])
                xt.append(t)
            wqt, wkt, wvt = [], [], []
            for dc in range(DCH):
                for lst, src, nm in ((wqt, wq, "wq"), (wkt, wk, "wk"), (wvt, wv, "wv")):
                    t = wpool.tile([128, FL], BF16, name=f"{nm}{dc}")
                    nc.sync.dma_start(out=t, in_=src.ap()[dc * 128:(dc + 1) * 128, :])
                    lst.append(t)
            wot = []
            for fc in range(2):
                t = wpool.tile([128, D], BF16, name=f"wo{fc}")
                nc.sync.dma_start(out=t, in_=wo.ap()[fc * 128:(fc + 1) * 128, :])
                wot.append(t)
            bq_sb = wpool.tile([128, 2], FP32, name="bq_sb")
            nc.sync.dma_start(out=bq_sb, in_=bqc.ap())
            bk_sb = wpool.tile([128, 2], FP32, name="bk_sb")
            nc.sync.dma_start(out=bk_sb, in_=bkc.ap())
            bv_sb = wpool.tile([128, FL], FP32, name="bv_sb")
            nc.sync.dma_start(out=bv_sb, in_=bvb.ap())
            mb_sb = wpool.tile([128, N_JC], FP32, name="mb_sb")
            nc.sync.dma_start(out=mb_sb, in_=mb.ap())

            # ones column at partition 64 for the recip broadcast matmul
            ones65 = wpool.tile([65, 64], BF16, name="ones65")
            nc.vector.memset(ones65[64:65, :], 1.0)

            # ---- persistent activations ----
            # qT/kT: tile p holds features [128p,128p+128) = heads 2p,2p+1
            qt = [apool.tile([128, S], BF16, name=f"qt{p}") for p in range(2)]
            kt = [apool.tile([128, S], BF16, name=f"kt{p}") for p in range(2)]
            # v natural: tile sc = rows [128sc,128sc+128), layout (128, 4 heads, 65)
            vt = [apool.tile([128, HL, 65], BF16, name=f"vt{sc}") for sc in range(N_JC)]
            # normalized attention output, transposed: (features, S)
            at = [apool.tile([128, S], BF16, name=f"at{p}") for p in range(2)]

            def qk_proj(pair, dst, w_tiles, bias_sb):
                """qT/kT for one head-pair: dst[:, s] = W[f,:] @ x[s,:] + b."""
                fc = pair
                for sc in range(N_SC):
                    ps = prpool.tile([128, SC], FP32, name="ps", tag="ps")
                    for dc in range(DCH):
                        nc.tensor.matmul(
                            ps,
                            lhsT=w_tiles[dc][:, fc * 128:(fc + 1) * 128],
                            rhs=xt[dc][:, sc * SC:(sc + 1) * SC],
                            start=(dc == 0),
                            stop=(dc == DCH - 1),
                        )
                    nc.vector.tensor_scalar_add(
                        dst[:, sc * SC:(sc + 1) * SC], ps, bias_sb[:, fc:fc + 1]
                    )

            def v_proj(sc):
                """v rows [128sc,128sc+128) for all 4 local heads + ones col."""
                ps = prpool.tile([128, FL], FP32, name="ps", tag="ps")
                for dc in range(DCH):
                    nc.tensor.matmul(
                        ps,
                        lhsT=xt[dc][:, sc * JC:(sc + 1) * JC],
                        rhs=wvt[dc],
                        start=(dc == 0),
                        stop=(dc == DCH - 1),
                    )
                nc.vector.tensor_add(
                    vt[sc][:, :, 0:64],
                    ps.rearrange("p (h d) -> p h d", h=HL),
                    bv_sb.rearrange("p (h d) -> p h d", h=HL),
                )
                nc.vector.memset(vt[sc][:, :, 64:65], 1.0)

            def attention(pair):
                """Full attention for heads (2*pair, 2*pair+1)."""
                for ic in range(N_SC):
                    i_sl = slice(ic * SC, (ic + 1) * SC)
                    outA = aopool.tile([65, SC], FP32, name="outA", tag="ao")
                    outB = aopool.tile([65, SC], FP32, name="outB", tag="ao")
                    for jc in range(N_JC):
                        sc_ps = scpool.tile([128, 2 * SC], FP32, name="sc_ps")
                        # scoresT = k @ q.T, two heads row-tiled (K=64 each)
                        nc.tensor.matmul(
                            sc_ps[:, 0:SC],
                            lhsT=kt[pair][0:64, jc * JC:(jc + 1) * JC],
                            rhs=qt[pair][0:64, i_sl],
                        )
                        nc.tensor.matmul(
                            sc_ps[:, SC:2 * SC],
                            lhsT=kt[pair][64:128, jc * JC:(jc + 1) * JC],
                            rhs=qt[pair][64:128, i_sl],
                        )
                        ex = epool.tile([128, 2 * SC], BF16, name="ex")
                        nc.scalar.activation(
                            ex, sc_ps, mybir.ActivationFunctionType.Exp,
                            bias=mb_sb[:, jc:jc + 1], scale=1.0 / np.sqrt(DH),
                        )
                        nc.tensor.matmul(
                            outA, lhsT=vt[jc][:, 2 * pair, :], rhs=ex[:, 0:SC],
                            start=(jc == 0), stop=(jc == N_JC - 1),
                        )
                        nc.tensor.matmul(
                            outB, lhsT=vt[jc][:, 2 * pair + 1, :], rhs=ex[:, SC:2 * SC],
                            start=(jc == 0), stop=(jc == N_JC - 1),
                        )
                    # normalize: rows 0..63 are attn@v, row 64 is sum(exp)
                    for half, ps_o in ((0, outA), (1, outB)):
                        rec = smpool.tile([65, SC], BF16, name="rec", tag="rec")
                        with nc.allow_low_precision(reason="softmax denom recip in bf16; 0.4% on weights is within tolerance"):
                            nc.vector.reciprocal(rec[64:65, :], ps_o[64:65, :])
                        bc = prpool.tile([64, SC], FP32, name="bc", tag="ps")
                        nc.tensor.matmul(bc, lhsT=ones65[64:65, :], rhs=rec[64:65, :])
                        # DVE can't read two PSUM operands: stage bc in SBUF
                        bc_sb = smpool.tile([64, SC], FP32, name="bc_sb", tag="bcsb")
                        nc.vector.tensor_copy(bc_sb, bc)
                        if half == 0:
                            nc.vector.tensor_mul(at[pair][0:64, i_sl], ps_o[0:64, :], bc_sb)
                        else:
                            stg = smpool.tile([64, SC], BF16, name="stg", tag="stg")
                            nc.vector.tensor_mul(stg, ps_o[0:64, :], bc_sb)
                            # shift to partitions 64..127 (DVE can't cross lanes)
                            nc.sync.dma_start(out=at[pair][64:128, i_sl], in_=stg)

            def out_proj(ic):
                """Partial output projection for s-window ic (needs both pairs)."""
                for ec in range(2):
                    for ss in range(SC // 128):
                        srow = ic * SC + ss * 128
                        po = prpool.tile([128, SC], FP32, name="po", tag="ps")
                        for fc in range(2):
                            nc.tensor.matmul(
                                po,
                                lhsT=at[fc][:, srow:srow + 128],
                                rhs=wot[fc][:, ec * SC:(ec + 1) * SC],
                                start=(fc == 0),
                                stop=(fc == 1),
                            )
                        stg = spool.tile([128, SC], FP32, name="ostg")
                        nc.vector.tensor_copy(stg, po)
                        nc.sync.dma_start(
                            out=out.ap()[srow:srow + 128, ec * SC:(ec + 1) * SC],
                            in_=stg,
                        )

            # ---- emission order (drives scheduling priority) ----
            qk_proj(0, kt[0], wkt, bk_sb)
            qk_proj(0, qt[0], wqt, bq_sb)
            for sc in range(N_JC):
                v_proj(sc)
            attention(0)
            qk_proj(1, kt[1], wkt, bk_sb)
            qk_proj(1, qt[1], wqt, bq_sb)
            attention(1)
            for ic in range(N_SC):
                out_proj(ic)

    nc.compile()
    return nc


_NC_CACHE = None


def _get_nc():
    global _NC_CACHE
    if _NC_CACHE is None:
        _NC_CACHE = build_kernel()
    return _NC_CACHE


def make_in_maps(inputs):
    x = np.asarray(inputs["x"], dtype=np.float32)
    mask = np.asarray(inputs["mask"])
    Wq = np.asarray(inputs["Wq"], dtype=np.float32)
    bq = np.asarray(inputs["bq"], dtype=np.float32)
    Wk = np.asarray(inputs["Wk"], dtype=np.float32)
    bk = np.asarray(inputs["bk"], dtype=np.float32)
    Wv = np.asarray(inputs["Wv"], dtype=np.float32)
    bv = np.asarray(inputs["bv"], dtype=np.float32)
    Wo = np.asarray(inputs["Wo"], dtype=np.float32)

    bf = ml_dtypes.bfloat16
    in_maps = []
    for c in range(NCORES):
        b = c // GROUPS
        g = c % GROUPS
        fs, fe = g * FL, (g + 1) * FL
        in_maps.append({
            "xT": np.ascontiguousarray(x[b].T).astype(bf),
            "wq": np.ascontiguousarray(Wq[fs:fe, :].T).astype(bf),
            "wk": np.ascontiguousarray(Wk[fs:fe, :].T).astype(bf),
            "wv": np.ascontiguousarray(Wv[fs:fe, :].T).astype(bf),
            "wo": np.ascontiguousarray(Wo[:, fs:fe].T).astype(bf),
            "bqc": np.ascontiguousarray(bq[fs:fe].reshape(2, 128).T),
            "bkc": np.ascontiguousarray(bk[fs:fe].reshape(2, 128).T),
            "bvb": np.tile(bv[fs:fe], (128, 1)).astype(np.float32),
            "mb": np.ascontiguousarray(
                np.where(mask[b] == 0, np.float32(-1e9), np.float32(0.0))
                .astype(np.float32).reshape(N_JC, 128).T
            ),
        })
    return in_maps


def kernel(x, mask, Wq, bq, Wk, bk, Wv, bv, Wo, bo):
    bo = np.asarray(bo, dtype=np.float32)
    nc = _get_nc()
    in_maps = make_in_maps(dict(x=x, mask=mask, Wq=Wq, bq=bq, Wk=Wk, bk=bk,
                                Wv=Wv, bv=bv, Wo=Wo, bo=bo))
    res = run_bass_kernel_spmd(nc, in_maps, core_ids=list(range(NCORES)))
    parts = [np.asarray(r["out"], dtype=np.float32) for r in res.results]
    full = np.empty((B, S, D), dtype=np.float32)
    for b in range(B):
        acc = parts[b * GROUPS].copy()
        for g in range(1, GROUPS):
            acc += parts[b * GROUPS + g]
        full[b] = acc + bo[None, :]
    return full
